# Initial kernel scaffold
#
"""DigitalTwinSSM Trainium2 kernel.

Data-parallel over batch: core b handles batch element b end-to-end.
Mamba block runs channel-major ([channel partitions, time free]); the
selective-scan recurrence maps onto VectorE tensor_tensor_scan (one
instruction per (d-block, state-index) pair, time along the free axis).
A_log has d-independent rows (A[d,n] = -(n+1)), so dA = exp(A_n * dt) is a
single scaled Exp per (d-block, n). The y = sum_n C_n * state_n reduction
runs on TensorE as bf16 identity-matmul accumulation into PSUM.
CDSP is reduced to per-core sufficient statistics (S_ha, sum_h, sum_h2)
combined with host-side a-statistics.
"""

import numpy as np
from contextlib import ExitStack

import concourse.bass as bass
import concourse.mybir as mybir
import concourse.tile as tile
from concourse.tile import ScopedClock
from concourse.bass_utils import run_bass_kernel_spmd
from concourse.kernels.tile_matmul import make_identity

f32 = mybir.dt.float32
f32r = mybir.dt.float32r
bf16 = mybir.dt.bfloat16
AF = mybir.ActivationFunctionType
ALU = mybir.AluOpType

B, T = 8, 1024
INPUT_DIM, TREAT_DIM, HIDDEN, OUT_DIM = 32, 8, 512, 32
D_STATE, D_CONV = 16, 4
D_INNER = 1024
DT_RANK = 32
TOTAL_IN = INPUT_DIM * 3 + TREAT_DIM  # 104
LN_EPS = 1e-5
N_CORES = 8
NJ = D_INNER // 128  # 8 d-blocks
NH = HIDDEN // 128   # 4 hidden blocks

MAX_WAITS = 1


class SplitWaitTileContext(tile.TileContext):
    """Enforce <=1 semaphore wait per instruction (walrus TPB limit here)."""

    def _split_waits(self, inst):
        si = inst.sync_info
        if si is None:
            return None
        waits = list(si.on_wait)
        if len(waits) <= MAX_WAITS:
            return None
        si.on_wait.clear()
        for w in waits[:MAX_WAITS]:
            si.on_wait.append(w)
        nops = []
        for i in range(MAX_WAITS, len(waits), MAX_WAITS):
            nop = mybir.InstNoOp(name=f"wsplit-{self.nc.next_id()}", ins=[], outs=[])
            nop.engine = inst.engine
            nop.sync_info = mybir.SyncInfo(
                on_wait=list(waits[i : i + MAX_WAITS]), on_update=[]
            )
            nops.append(nop)
        return nops

    def _commit_instruction(self, inst, lazy_reg_writes: bool = True):
        nops = self._split_waits(inst)
        if nops:
            for nop in nops:
                super()._commit_instruction(nop, lazy_reg_writes)
        return super()._commit_instruction(inst, lazy_reg_writes)

    def _drain_and_barrier(self, tick_clock, wait_clock):
        nc = self.nc
        drain_inst = nc.sync.drain()
        wait_clock.add_sem_waits(
            drain_inst.ins, ScopedClock({None: tick_clock.global_clock})
        )
        waits = list(drain_inst.ins.sync_info.on_wait)
        if len(waits) > MAX_WAITS:
            drain_inst.ins.sync_info.on_wait.clear()
            for w in waits[:MAX_WAITS]:
                drain_inst.ins.sync_info.on_wait.append(w)
            for i in range(MAX_WAITS, len(waits), MAX_WAITS):
                n = nc.sync.nop(nofuse=True)
                n.ins.sync_info = mybir.SyncInfo(
                    on_wait=list(waits[i : i + MAX_WAITS]), on_update=[]
                )
        nc.all_engine_barrier()
        assert self.sems is not None
        popped = nc._tile_sem_poison_stack.pop()
        assert popped is self._sem_poison
        nc.clear_and_free_semaphores(list(self.sems.allocated().values()))
        nc.all_engine_barrier()


def _build_program(a_vals):
    """Build the SPMD Bass program. a_vals: 16 floats, A[n] = -exp(A_log[0,n])."""
    nc = bass.Bass("TRN2", target_bir_lowering=False, debug=False,
                   num_devices=N_CORES)

    def din(name, shape):
        return nc.dram_tensor(name, shape, f32, kind="ExternalInput").ap()

    def dout(name, shape):
        return nc.dram_tensor(name, shape, f32, kind="ExternalOutput").ap()

    inp = din("inp", [T, TOTAL_IN])
    W_in = din("W_in", [TOTAL_IN, HIDDEN])
    b_in = din("b_in", [HIDDEN])
    in_proj_w = din("in_proj_w", [HIDDEN, 2 * D_INNER])
    conv_w = din("conv_w", [D_INNER, D_CONV])
    conv_b = din("conv_b", [D_INNER])
    x_proj_w = din("x_proj_w", [D_INNER, DT_RANK + 2 * D_STATE])
    dt_proj_w = din("dt_proj_w", [DT_RANK, D_INNER])
    dt_proj_b = din("dt_proj_b", [D_INNER])
    D_param = din("D_param", [D_INNER])
    out_proj_w = din("out_proj_w", [D_INNER, HIDDEN])
    ln_g = din("ln_g", [HIDDEN])
    ln_b = din("ln_b", [HIDDEN])
    mean_w = din("mean_w", [HIDDEN, OUT_DIM])
    mean_b = din("mean_b", [OUT_DIM])
    logvar_w = din("logvar_w", [HIDDEN, OUT_DIM])
    logvar_b = din("logvar_b", [OUT_DIM])
    hawkes_w = din("hawkes_w", [HIDDEN, TREAT_DIM])
    hawkes_bias = din("hawkes_bias", [TREAT_DIM])  # hawkes_base + hawkes_b

    y_pred_o = dout("y_pred", [T, OUT_DIM])
    y_var_o = dout("y_var", [T, OUT_DIM])
    hawkes_o = dout("hawkes", [T, TREAT_DIM])
    h_o = dout("h", [T, HIDDEN])
    S_ha_o = dout("S_ha", [TREAT_DIM, HIDDEN])
    s_h_o = dout("s_h", [HIDDEN])
    q_h_o = dout("q_h", [HIDDEN])

    # internal DRAM scratch
    bc_rows = nc.dram_tensor("bc_rows", [2 * D_STATE, T], bf16).ap()
    ln_rows = nc.dram_tensor("ln_rows", [2, T], f32).ap()

    with SplitWaitTileContext(nc) as tc, ExitStack() as ctx:
        pool1 = ctx.enter_context(tc.tile_pool(name="const", bufs=1))

        # ---- constants / biases ----
        ident_b = pool1.tile([128, 128], bf16)
        make_identity(nc, ident_b)
        ident_f = pool1.tile([128, 128], f32)
        make_identity(nc, ident_f)
        ones_r = pool1.tile([128, 1], f32r)
        nc.vector.memset(ones_r[:], 1.0)
        one_c = pool1.tile([1, 1], f32)
        nc.vector.memset(one_c[:], 1.0)
        eps_c = pool1.tile([1, 1], f32)
        nc.vector.memset(eps_c[:], LN_EPS)

        b_in_t = pool1.tile([128, NH], f32)
        for m in range(NH):
            nc.sync.dma_start(b_in_t[:, m : m + 1], b_in[m * 128 : (m + 1) * 128, None])
        convb_t = pool1.tile([128, NJ], f32)
        dtb_t = pool1.tile([128, NJ], f32)
        Dp_t = pool1.tile([128, NJ], f32)
        for j in range(NJ):
            sl = slice(j * 128, (j + 1) * 128)
            nc.sync.dma_start(convb_t[:, j : j + 1], conv_b[sl, None])
            nc.sync.dma_start(dtb_t[:, j : j + 1], dt_proj_b[sl, None])
            nc.sync.dma_start(Dp_t[:, j : j + 1], D_param[sl, None])
        lng_t = pool1.tile([128, NH], f32)
        lnb_t = pool1.tile([128, NH], f32)
        for m in range(NH):
            sl = slice(m * 128, (m + 1) * 128)
            nc.sync.dma_start(lng_t[:, m : m + 1], ln_g[sl, None])
            nc.sync.dma_start(lnb_t[:, m : m + 1], ln_b[sl, None])
        meanb_t = pool1.tile([32, 1], f32)
        nc.sync.dma_start(meanb_t[:], mean_b[:, None])
        logvb_t = pool1.tile([32, 1], f32)
        nc.sync.dma_start(logvb_t[:], logvar_b[:, None])
        hawkb_t = pool1.tile([8, 1], f32)
        nc.sync.dma_start(hawkb_t[:], hawkes_bias[:, None])
        convw_t = pool1.tile([128, NJ * D_CONV], f32)
        for j in range(NJ):
            nc.sync.dma_start(
                convw_t[:, j * D_CONV : (j + 1) * D_CONV],
                conv_w[j * 128 : (j + 1) * 128, :],
            )

        # ---- phase 0: transpose inp -> inpT [104, T] (f32r) ----
        tp_pool = ctx.enter_context(tc.tile_pool(name="tp", bufs=3))
        tp_psum = ctx.enter_context(tc.tile_pool(name="tp_ps", bufs=3, space="PSUM"))
        inpT = pool1.tile([TOTAL_IN, T], f32r)
        for i in range(T // 128):
            it = tp_pool.tile([128, TOTAL_IN], f32, tag="inp_in")
            nc.sync.dma_start(it[:], inp[i * 128 : (i + 1) * 128, :])
            ps = tp_psum.tile([TOTAL_IN, 128], f32, tag="inp_ps")
            nc.tensor.transpose(ps[:], it[:], ident_f[:])
            nc.scalar.copy(inpT[:, i * 128 : (i + 1) * 128], ps[:])

        # ---- phase 1: projT = (inp @ W_in + b_in).T  [512, T] f32r ----
        wpool = ctx.enter_context(tc.tile_pool(name="wts", bufs=2))
        gem_ps = ctx.enter_context(tc.tile_pool(name="gem_ps", bufs=4, space="PSUM"))
        W_in_f = wpool.tile([TOTAL_IN, HIDDEN], f32, tag="win")
        nc.sync.dma_start(W_in_f[:], W_in[:])
        W_in_r = wpool.tile([TOTAL_IN, HIDDEN], f32r, tag="win_r")
        nc.gpsimd.tensor_copy(W_in_r[:], W_in_f[:])
        projT = pool1.tile([HIDDEN, T], f32r)  # 4 x [128, T] stacked on partitions? no: [512,T] invalid
        # NOTE: SBUF tiles max 128 partitions; store as [128, NH*T]
        # projT[c, t] at tile[c % 128, (c//128)*T + t]
        del projT
        projT = pool1.tile([128, NH * T], f32r)
        for m in range(NH):
            for half in range(2):
                ps = gem_ps.tile([128, 512], f32, tag="g1")
                nc.tensor.matmul(
                    ps[:],
                    W_in_r[:, m * 128 : (m + 1) * 128],
                    inpT[:, half * 512 : (half + 1) * 512],
                    start=True, stop=True,
                )
                nc.scalar.activation(
                    projT[:, m * T + half * 512 : m * T + (half + 1) * 512],
                    ps[:], AF.Identity, bias=b_in_t[:, m : m + 1],
                )

        # ---- phase 2 (loop 1 over j): xz GEMM, conv, silu, x_dbl accum ----
        ipw_pool = ctx.enter_context(tc.tile_pool(name="ipw", bufs=2))
        ipw_r = []
        for k in range(NH):
            wf = ipw_pool.tile([128, 2 * D_INNER], f32, tag="ipw_f")
            nc.sync.dma_start(wf[:], in_proj_w[k * 128 : (k + 1) * 128, :])
            wr = wpool.tile([128, 2 * D_INNER], f32r, tag=f"ipw_r{k}")
            nc.gpsimd.tensor_copy(wr[:], wf[:])
            ipw_r.append(wr)

        xpw_f = ipw_pool.tile([128, NJ * (DT_RANK + 2 * D_STATE)], f32, tag="xpw_f")
        for k in range(NJ):
            nc.sync.dma_start(
                xpw_f[:, k * 64 : (k + 1) * 64], x_proj_w[k * 128 : (k + 1) * 128, :]
            )
        xpw_r = wpool.tile([128, NJ * 64], f32r, tag="xpw_r")
        nc.gpsimd.tensor_copy(xpw_r[:], xpw_f[:])

        uT = pool1.tile([128, NJ * T], f32r)      # silu(conv(xi))
        zsT = pool1.tile([128, NJ * T], bf16)     # silu(z)
        xdbl_ps = ctx.enter_context(tc.tile_pool(name="xdbl_ps", bufs=1, space="PSUM"))
        xdbl_psum = xdbl_ps.tile([64, T], f32)

        conv_pool = ctx.enter_context(tc.tile_pool(name="conv", bufs=3))

        for j in range(NJ):
            # xi_j
            xi_pad = conv_pool.tile([128, T + 3], f32, tag="xi_pad")
            nc.vector.memset(xi_pad[:, 0:3], 0.0)
            for half in range(2):
                ps = gem_ps.tile([128, 512], f32, tag="xz")
                for k in range(NH):
                    nc.tensor.matmul(
                        ps[:],
                        ipw_r[k][:, j * 128 : (j + 1) * 128],
                        projT[:, k * T + half * 512 : k * T + (half + 1) * 512],
                        start=(k == 0), stop=(k == NH - 1),
                    )
                nc.scalar.copy(xi_pad[:, 3 + half * 512 : 3 + (half + 1) * 512], ps[:])
            # z_j -> silu -> bf16
            for half in range(2):
                ps = gem_ps.tile([128, 512], f32, tag="xz")
                for k in range(NH):
                    nc.tensor.matmul(
                        ps[:],
                        ipw_r[k][:, (NJ + j) * 128 : (NJ + j + 1) * 128],
                        projT[:, k * T + half * 512 : k * T + (half + 1) * 512],
                        start=(k == 0), stop=(k == NH - 1),
                    )
                nc.scalar.activation(
                    zsT[:, j * T + half * 512 : j * T + (half + 1) * 512],
                    ps[:], AF.Silu,
                )
            # causal depthwise conv (width 4) + bias + silu -> uT
            xc = conv_pool.tile([128, T], f32, tag="xc")
            nc.vector.tensor_scalar(
                xc[:], xi_pad[:, 0:T], convw_t[:, j * D_CONV : j * D_CONV + 1], None,
                ALU.mult,
            )
            for k in range(1, D_CONV):
                xc2 = conv_pool.tile([128, T], f32, tag="xc")
                nc.vector.scalar_tensor_tensor(
                    xc2[:], xi_pad[:, k : k + T],
                    convw_t[:, j * D_CONV + k : j * D_CONV + k + 1],
                    xc[:], ALU.mult, ALU.add,
                )
                xc = xc2
            nc.scalar.activation(
                uT[:, j * T : (j + 1) * T], xc[:], AF.Silu,
                bias=convb_t[:, j : j + 1],
            )
            # x_dbl accumulation: xdbl += x_proj_w[j].T @ u_j
            for half in range(2):
                nc.tensor.matmul(
                    xdbl_psum[:, half * 512 : (half + 1) * 512],
                    xpw_r[:, j * 64 : (j + 1) * 64],
                    uT[:, j * T + half * 512 : j * T + (half + 1) * 512],
                    start=(j == 0), stop=(j == NJ - 1),
                )

        # evict x_dbl: rows 0:32 dt_r (f32r), rows 32:64 B/C -> bf16 -> DRAM
        xdbl_sb = pool1.tile([64, T], f32r)
        nc.scalar.copy(xdbl_sb[:], xdbl_psum[:])
        bc_sb = pool1.tile([32, T], bf16)
        nc.scalar.copy(bc_sb[:], xdbl_psum[32:64, :])
        nc.sync.dma_start(bc_rows[:], bc_sb[:])

        # dt_proj_w [32, 1024] f32 -> f32r
        dtw_f = ipw_pool.tile([32, D_INNER], f32, tag="dtw_f")
        nc.sync.dma_start(dtw_f[:], dt_proj_w[:])
        dtw_r = wpool.tile([32, D_INNER], f32r, tag="dtw_r")
        nc.gpsimd.tensor_copy(dtw_r[:], dtw_f[:])

        # ---- phase 3 (loop 2 over j): dt, scan over 16 states, gating ----
        ygT = pool1.tile([128, NJ * T], f32r)
        scan_pool = ctx.enter_context(tc.tile_pool(name="scan", bufs=3))
        bc_pool = ctx.enter_context(tc.tile_pool(name="bc", bufs=1))
        yacc_ps = ctx.enter_context(tc.tile_pool(name="yacc", bufs=2, space="PSUM"))
        dt_ps = ctx.enter_context(tc.tile_pool(name="dt_ps", bufs=2, space="PSUM"))

        # broadcast B/C rows once: 32 tiles [128, T] bf16
        Bbc = bc_pool.tile([128, D_STATE * T], bf16)
        Cbc = bc_pool.tile([128, D_STATE * T], bf16)
        for n in range(D_STATE):
            nc.sync.dma_start(
                Bbc[:, n * T : (n + 1) * T], bc_rows[n, None, :].broadcast_to((128, T))
            )
            nc.sync.dma_start(
                Cbc[:, n * T : (n + 1) * T],
                bc_rows[D_STATE + n, None, :].broadcast_to((128, T)),
            )

        for j in range(NJ):
            # dt_pre = dt_proj_w.T @ dt_r   [128, T]
            dt_psum = dt_ps.tile([128, T], f32, tag="dtg")
            for half in range(2):
                nc.tensor.matmul(
                    dt_psum[:, half * 512 : (half + 1) * 512],
                    dtw_r[:, j * 128 : (j + 1) * 128],
                    xdbl_sb[0:DT_RANK, half * 512 : (half + 1) * 512],
                    start=True, stop=True,
                )
            # softplus: dt = ln(exp(dt_pre + b) + 1)
            e_t = scan_pool.tile([128, T], f32, tag="sp_e")
            nc.scalar.activation(e_t[:], dt_psum[:], AF.Exp, bias=dtb_t[:, j : j + 1])
            dt_t = scan_pool.tile([128, T], f32, tag="dt")
            nc.scalar.activation(dt_t[:], e_t[:], AF.Ln, bias=one_c[0:1, 0:1].broadcast_to((128, 1)))
            # dtu = dt * u (bf16)
            dtu_t = scan_pool.tile([128, T], bf16, tag="dtu")
            nc.vector.tensor_tensor(
                dtu_t[:], dt_t[:], uT[:, j * T : (j + 1) * T].bitcast(f32), ALU.mult
            )

            yacc = yacc_ps.tile([128, T], f32, tag="yacc")
            for n in range(D_STATE):
                dA = scan_pool.tile([128, T], bf16, tag="dA")
                nc.scalar.activation(dA[:], dt_t[:], AF.Exp, scale=float(a_vals[n]))
                bterm = scan_pool.tile([128, T], bf16, tag="bterm")
                nc.vector.tensor_tensor(
                    bterm[:], dtu_t[:], Bbc[:, n * T : (n + 1) * T], ALU.mult
                )
                st = scan_pool.tile([128, T], bf16, tag="state")
                nc.vector.tensor_tensor_scan(
                    st[:], dA[:], bterm[:], 0.0, ALU.mult, ALU.add
                )
                ym = scan_pool.tile([128, T], bf16, tag="ym")
                nc.vector.tensor_tensor(
                    ym[:], st[:], Cbc[:, n * T : (n + 1) * T], ALU.mult
                )
                for half in range(2):
                    nc.tensor.matmul(
                        yacc[:, half * 512 : (half + 1) * 512],
                        ident_b[:],
                        ym[:, half * 512 : (half + 1) * 512],
                        start=(n == 0), stop=(n == D_STATE - 1),
                    )
            # y2 = u * D + yacc  (bf16), y3 = y2 * silu(z) -> f32r
            y2 = scan_pool.tile([128, T], bf16, tag="y2")
            nc.vector.scalar_tensor_tensor(
                y2[:], uT[:, j * T : (j + 1) * T].bitcast(f32), Dp_t[:, j : j + 1],
                yacc[:], ALU.mult, ALU.add,
            )
            nc.vector.tensor_tensor(
                ygT[:, j * T : (j + 1) * T], y2[:], zsT[:, j * T : (j + 1) * T],
                ALU.mult,
            )

        # ---- phase 4: out_proj -> hpreT [512, T] f32r ----
        opw_r = []
        for k in range(NJ):
            wf = ipw_pool.tile([128, HIDDEN], f32, tag="opw_f")
            nc.sync.dma_start(wf[:], out_proj_w[k * 128 : (k + 1) * 128, :])
            wr = wpool.tile([128, HIDDEN], f32r, tag=f"opw_r{k}")
            nc.gpsimd.tensor_copy(wr[:], wf[:])
            opw_r.append(wr)
        hpreT = pool1.tile([128, NH * T], f32r)
        for m in range(NH):
            for half in range(2):
                ps = gem_ps.tile([128, 512], f32, tag="op")
                for k in range(NJ):
                    nc.tensor.matmul(
                        ps[:],
                        opw_r[k][:, m * 128 : (m + 1) * 128],
                        ygT[:, k * T + half * 512 : k * T + (half + 1) * 512],
                        start=(k == 0), stop=(k == NJ - 1),
                    )
                nc.scalar.copy(
                    hpreT[:, m * T + half * 512 : m * T + (half + 1) * 512], ps[:]
                )

        # ---- phase 5: layernorm stats via ones-matmul ----
        ln_ps = ctx.enter_context(tc.tile_pool(name="ln_ps", bufs=2, space="PSUM"))
        hsq = pool1.tile([128, NH * T], f32r)
        for m in range(NH):
            nc.scalar.activation(
                hsq[:, m * T : (m + 1) * T], hpreT[:, m * T : (m + 1) * T], AF.Square
            )
        sum_ps = ln_ps.tile([1, T], f32, tag="s")
        sq_ps = ln_ps.tile([1, T], f32, tag="sq")
        for half in range(2):
            for m in range(NH):
                nc.tensor.matmul(
                    sum_ps[:, half * 512 : (half + 1) * 512],
                    ones_r[:, 0:1],
                    hpreT[:, m * T + half * 512 : m * T + (half + 1) * 512],
                    start=(m == 0), stop=(m == NH - 1),
                )
                nc.tensor.matmul(
                    sq_ps[:, half * 512 : (half + 1) * 512],
                    ones_r[:, 0:1],
                    hsq[:, m * T + half * 512 : m * T + (half + 1) * 512],
                    start=(m == 0), stop=(m == NH - 1),
                )
        mu = pool1.tile([1, T], f32)
        nc.vector.tensor_scalar(mu[:], sum_ps[:], 1.0 / HIDDEN, None, ALU.mult)
        e2 = pool1.tile([1, T], f32)
        nc.vector.tensor_scalar(e2[:], sq_ps[:], 1.0 / HIDDEN, None, ALU.mult)
        musq = pool1.tile([1, T], f32)
        nc.scalar.activation(musq[:], mu[:], AF.Square)
        var = pool1.tile([1, T], f32)
        nc.vector.tensor_tensor(var[:], e2[:], musq[:], ALU.subtract)
        lnv = pool1.tile([1, T], f32)
        nc.scalar.activation(lnv[:], var[:], AF.Ln, bias=eps_c[:])
        isd = pool1.tile([1, T], f32)
        nc.scalar.activation(isd[:], lnv[:], AF.Exp, scale=-0.5)
        m2 = pool1.tile([1, T], f32)
        nc.vector.tensor_tensor(m2[:], mu[:], isd[:], ALU.mult)
        ln2 = pool1.tile([2, T], f32)
        nc.vector.tensor_copy(ln2[0:1, :], isd[:])
        nc.vector.tensor_copy(ln2[1:2, :], m2[:])
        nc.sync.dma_start(ln_rows[:], ln2[:])
        isd_b = pool1.tile([128, T], f32)
        nc.sync.dma_start(isd_b[:], ln_rows[0, None, :].broadcast_to((128, T)))
        m2_b = pool1.tile([128, T], f32)
        nc.sync.dma_start(m2_b[:], ln_rows[1, None, :].broadcast_to((128, T)))

        # ---- phase 6: h_ln = (h*isd - m2) * g + b   [channel-major, f32r] ----
        hlnT = pool1.tile([128, NH * T], f32r)
        ln_work = ctx.enter_context(tc.tile_pool(name="lnw", bufs=3))
        for m in range(NH):
            t1 = ln_work.tile([128, T], f32, tag="t1")
            nc.vector.tensor_tensor(
                t1[:], hpreT[:, m * T : (m + 1) * T].bitcast(f32), isd_b[:], ALU.mult
            )
            t2 = ln_work.tile([128, T], f32, tag="t2")
            nc.vector.tensor_tensor(t2[:], t1[:], m2_b[:], ALU.subtract)
            nc.vector.tensor_scalar(
                hlnT[:, m * T : (m + 1) * T], t2[:],
                lng_t[:, m : m + 1], lnb_t[:, m : m + 1], ALU.mult, op1=ALU.add,
            )

        # ---- phase 7: heads (channel-major GEMMs) ----
        hw_pool = ctx.enter_context(tc.tile_pool(name="hw", bufs=2))
        head_ps = ctx.enter_context(tc.tile_pool(name="head_ps", bufs=2, space="PSUM"))

        def head_gemm(w_ap, odim, tag):
            wf = hw_pool.tile([128, NH * odim], f32, tag=f"{tag}_f")
            for k in range(NH):
                nc.sync.dma_start(
                    wf[:, k * odim : (k + 1) * odim], w_ap[k * 128 : (k + 1) * 128, :]
                )
            wr = hw_pool.tile([128, NH * odim], f32r, tag=f"{tag}_r")
            nc.gpsimd.tensor_copy(wr[:], wf[:])
            ps = head_ps.tile([odim, T], f32, tag=tag)
            for half in range(2):
                for k in range(NH):
                    nc.tensor.matmul(
                        ps[:, half * 512 : (half + 1) * 512],
                        wr[:, k * odim : (k + 1) * odim],
                        hlnT[:, k * T + half * 512 : k * T + (half + 1) * 512],
                        start=(k == 0), stop=(k == NH - 1),
                    )
            return ps

        yp_ps = head_gemm(mean_w, OUT_DIM, "yp")
        ypredT = pool1.tile([32, T], f32)
        nc.scalar.activation(ypredT[:], yp_ps[:], AF.Identity, bias=meanb_t[:])

        yv_ps = head_gemm(logvar_w, OUT_DIM, "yv")
        yv_e = pool1.tile([32, T], f32)
        nc.scalar.activation(yv_e[:], yv_ps[:], AF.Exp, bias=logvb_t[:])
        yv_sp = pool1.tile([32, T], f32)
        nc.scalar.activation(
            yv_sp[:], yv_e[:], AF.Ln, bias=one_c[0:1, 0:1].broadcast_to((32, 1))
        )
        yvarT = pool1.tile([32, T], f32)
        nc.vector.tensor_scalar(yvarT[:], yv_sp[:], 1e-4, None, ALU.add)

        hk_ps = head_gemm(hawkes_w, TREAT_DIM, "hk")
        hk_e = pool1.tile([8, T], f32)
        nc.scalar.activation(hk_e[:], hk_ps[:], AF.Exp, bias=hawkb_t[:])
        hawkesT = pool1.tile([8, T], f32)
        nc.scalar.activation(
            hawkesT[:], hk_e[:], AF.Ln, bias=one_c[0:1, 0:1].broadcast_to((8, 1))
        )

        # ---- phase 8: transposes to token-major + outputs ----
        out_ps = ctx.enter_context(tc.tile_pool(name="out_ps", bufs=3, space="PSUM"))
        out_sb = ctx.enter_context(tc.tile_pool(name="out_sb", bufs=3))
        # h output: transpose hlnT -> [T, 512]
        for i in range(T // 128):
            hps = out_ps.tile([128, HIDDEN], f32r, tag="h_ps")
            for m in range(NH):
                nc.tensor.transpose(
                    hps[:, m * 128 : (m + 1) * 128],
                    hlnT[:, m * T + i * 128 : m * T + (i + 1) * 128],
                    ident_f[:],
                )
            hsb = out_sb.tile([128, HIDDEN], f32, tag="h_sb")
            nc.scalar.copy(hsb[:], hps[:].bitcast(f32))
            nc.sync.dma_start(h_o[i * 128 : (i + 1) * 128, :], hsb[:])
        # y_pred / y_var / hawkes outputs
        for i in range(T // 128):
            pps = out_ps.tile([128, 72], f32, tag="y_ps")
            nc.tensor.transpose(
                pps[:, 0:32], ypredT[:, i * 128 : (i + 1) * 128], ident_f[:]
            )
            nc.tensor.transpose(
                pps[:, 32:64], yvarT[:, i * 128 : (i + 1) * 128], ident_f[:]
            )
            nc.tensor.transpose(
                pps[:, 64:72], hawkesT[:, i * 128 : (i + 1) * 128], ident_f[:]
            )
            psb = out_sb.tile([128, 72], f32, tag="y_sb")
            nc.scalar.copy(psb[:], pps[:])
            nc.sync.dma_start(y_pred_o[i * 128 : (i + 1) * 128, :], psb[:, 0:32])
            nc.sync.dma_start(y_var_o[i * 128 : (i + 1) * 128, :], psb[:, 32:64])
            nc.sync.dma_start(hawkes_o[i * 128 : (i + 1) * 128, :], psb[:, 64:72])

        # ---- phase 9: CDSP partials ----
        # h_f = h_ln[::4] token-major [256, 512] f32r via strided transposes
        hf_sb = []
        for i in range(2):  # two 128-token tiles of the 256 subsampled tokens
            fps = out_ps.tile([128, HIDDEN], f32r, tag="hf_ps")
            for m in range(NH):
                src = hlnT[:, m * T + i * 512 : m * T + (i + 1) * 512 : 4]
                nc.tensor.transpose(fps[:, m * 128 : (m + 1) * 128], src, ident_f[:])
            fsb = out_sb.tile([128, HIDDEN], f32r, tag="hf_sb")
            nc.scalar.copy(fsb[:], fps[:])
            hf_sb.append(fsb)
        # a_f = inp[::4, 96:104] -> [256, 8] f32r
        af_f = out_sb.tile([128, 2 * TREAT_DIM], f32, tag="af_f")
        for i in range(2):
            nc.sync.dma_start(
                af_f[:, i * TREAT_DIM : (i + 1) * TREAT_DIM],
                inp[i * 512 : (i + 1) * 512 : 4, INPUT_DIM * 3 :],
            )
        af_r = out_sb.tile([128, 2 * TREAT_DIM], f32r, tag="af_r")
        nc.gpsimd.tensor_copy(af_r[:], af_f[:])
        sha_ps = out_ps.tile([TREAT_DIM, HIDDEN], f32, tag="sha")
        for i in range(2):
            nc.tensor.matmul(
                sha_ps[:], af_r[:, i * TREAT_DIM : (i + 1) * TREAT_DIM], hf_sb[i][:],
                start=(i == 0), stop=(i == 1),
            )
        sha_sb = out_sb.tile([TREAT_DIM, HIDDEN], f32, tag="sha_sb")
        nc.scalar.copy(sha_sb[:], sha_ps[:])
        nc.sync.dma_start(S_ha_o[:], sha_sb[:])
        # s_h, q_h per channel over subsampled tokens
        dump = out_sb.tile([128, T // 4], f32, tag="dump")
        sh_acc = out_sb.tile([128, NH], f32, tag="sh")
        qh_acc = out_sb.tile([128, NH], f32, tag="qh")
        for m in range(NH):
            nc.scalar.activation(
                dump[:], hlnT[:, m * T : (m + 1) * T : 4], AF.Identity,
                accum_out=sh_acc[:, m : m + 1],
            )
            nc.scalar.activation(
                dump[:], hlnT[:, m * T : (m + 1) * T : 4], AF.Square,
                accum_out=qh_acc[:, m : m + 1],
            )
        for m in range(NH):
            nc.sync.dma_start(s_h_o[m * 128 : (m + 1) * 128], sh_acc[:, m : m + 1])
            nc.sync.dma_start(q_h_o[m * 128 : (m + 1) * 128], qh_acc[:, m : m + 1])

    return nc


_PROGRAM_CACHE = {}


def kernel(**inputs):
    x_seq = np.asarray(inputs["x_seq"], np.float32)
    mask_seq = np.asarray(inputs["mask_seq"], np.float32)
    dt_seq = np.asarray(inputs["dt_seq"], np.float32)
    a_seq = np.asarray(inputs["a_seq"], np.float32)
    A_log = np.asarray(inputs["A_log"], np.float32)

    inp_all = np.concatenate([x_seq, mask_seq, dt_seq, a_seq], axis=-1)  # [B,T,104]
    a_vals = tuple((-np.exp(A_log[0])).astype(np.float64).tolist())

    key = a_vals
    if key not in _PROGRAM_CACHE:
        _PROGRAM_CACHE[key] = _build_program(a_vals)
    nc = _PROGRAM_CACHE[key]

    shared = {
        "W_in": inputs["W_in"], "b_in": inputs["b_in"],
        "in_proj_w": inputs["in_proj_w"],
        "conv_w": inputs["conv_w"], "conv_b": inputs["conv_b"],
        "x_proj_w": inputs["x_proj_w"],
        "dt_proj_w": inputs["dt_proj_w"], "dt_proj_b": inputs["dt_proj_b"],
        "D_param": inputs["D_param"], "out_proj_w": inputs["out_proj_w"],
        "ln_g": inputs["ln_g"], "ln_b": inputs["ln_b"],
        "mean_w": inputs["mean_w"], "mean_b": inputs["mean_b"],
        "logvar_w": inputs["logvar_w"], "logvar_b": inputs["logvar_b"],
        "hawkes_w": inputs["hawkes_w"],
        "hawkes_bias": np.asarray(inputs["hawkes_base"], np.float32)
        + np.asarray(inputs["hawkes_b"], np.float32),
    }
    shared = {k: np.ascontiguousarray(v, dtype=np.float32) for k, v in shared.items()}
    in_maps = [dict(shared, inp=np.ascontiguousarray(inp_all[b])) for b in range(B)]

    res = run_bass_kernel_spmd(nc, in_maps, list(range(N_CORES)))
    outs = res.results

    y_pred = np.stack([outs[b]["y_pred"] for b in range(B)])
    y_var = np.stack([outs[b]["y_var"] for b in range(B)])
    hawkes = np.stack([outs[b]["hawkes"] for b in range(B)])
    h = np.stack([outs[b]["h"] for b in range(B)])

    # CDSP from per-core sufficient statistics + host a-side stats
    S_ha = np.sum([outs[b]["S_ha"] for b in range(B)], axis=0).T  # [512, 8]
    s_h = np.sum([outs[b]["s_h"] for b in range(B)], axis=0)      # [512]
    q_h = np.sum([outs[b]["q_h"] for b in range(B)], axis=0)      # [512]
    a_f = a_seq[:, ::4, :].reshape(-1, TREAT_DIM).astype(np.float64)
    n_rows = a_f.shape[0]
    mu_h = s_h.astype(np.float64) / n_rows
    mu_a = a_f.mean(0)
    cross = S_ha.astype(np.float64) - n_rows * np.outer(mu_h, mu_a)
    nh2 = q_h.astype(np.float64) - n_rows * mu_h**2
    norm_h = np.sqrt(np.maximum(nh2, 0.0))
    a_c = a_f - mu_a
    norm_a = np.sqrt((a_c**2).sum(0))
    corr = cross / ((norm_h[:, None] + 1e-6) * (norm_a[None, :] + 1e-6))
    cdsp = np.float32((corr**2).sum() / (HIDDEN * TREAT_DIM))

    return (y_pred, y_var, hawkes, np.float32(cdsp), h)


# revision 12
# speedup vs baseline: 1.0665x; 1.0665x over previous
"""DigitalTwinSSM Trainium2 kernel.

Data-parallel over batch: core b handles batch element b end-to-end.
The Mamba block runs channel-major ([channel partitions, time free]); the
selective-scan recurrence maps onto VectorE tensor_tensor_scan (one
instruction per (d-block, state-index) pair, time along the free axis).
A_log rows are d-independent (A[d,n] = A[n]), so dA = exp(A[n] * dt) is a
single scaled Exp per (d-block, n). The y = sum_n C_n * state_n reduction
runs on TensorE as bf16 identity-matmul accumulation into PSUM.
CDSP reduces to per-core sufficient statistics (S_ha, sum_h, sum_h2)
combined with host-side a-statistics.
"""

import os
import numpy as np
from contextlib import ExitStack

import concourse.bass as bass
import concourse.mybir as mybir
import concourse.tile as tile
from concourse.tile import ScopedClock
from concourse.bass_utils import run_bass_kernel_spmd
from concourse.kernels.tile_matmul import make_identity

f32 = mybir.dt.float32
f32r = mybir.dt.float32r
bf16 = mybir.dt.bfloat16
AF = mybir.ActivationFunctionType
ALU = mybir.AluOpType

B, T = 8, 1024
INPUT_DIM, TREAT_DIM, HIDDEN, OUT_DIM = 32, 8, 512, 32
D_STATE, D_CONV = 16, 4
D_INNER = 1024
DT_RANK = 32
TOTAL_IN = INPUT_DIM * 3 + TREAT_DIM  # 104
LN_EPS = 1e-5
N_CORES = 8
NJ = D_INNER // 128  # 8 d-blocks
NH = HIDDEN // 128   # 4 hidden-channel blocks

MAX_WAITS = 1


class SplitWaitTileContext(tile.TileContext):
    """Enforce <=1 semaphore wait per instruction (walrus TPB limit here)."""

    def _split_waits(self, inst):
        si = inst.sync_info
        if si is None:
            return None
        waits = list(si.on_wait)
        if len(waits) <= MAX_WAITS:
            return None
        si.on_wait.clear()
        for w in waits[:MAX_WAITS]:
            si.on_wait.append(w)
        nops = []
        for i in range(MAX_WAITS, len(waits), MAX_WAITS):
            nop = mybir.InstNoOp(name=f"wsplit-{self.nc.next_id()}", ins=[], outs=[])
            nop.engine = inst.engine
            nop.sync_info = mybir.SyncInfo(
                on_wait=list(waits[i : i + MAX_WAITS]), on_update=[]
            )
            nops.append(nop)
        return nops

    def _commit_instruction(self, inst, lazy_reg_writes: bool = True):
        nops = self._split_waits(inst)
        if nops:
            for nop in nops:
                super()._commit_instruction(nop, lazy_reg_writes)
        return super()._commit_instruction(inst, lazy_reg_writes)

    def _drain_and_barrier(self, tick_clock, wait_clock):
        nc = self.nc
        drain_inst = nc.sync.drain()
        wait_clock.add_sem_waits(
            drain_inst.ins, ScopedClock({None: tick_clock.global_clock})
        )
        waits = list(drain_inst.ins.sync_info.on_wait)
        if len(waits) > MAX_WAITS:
            drain_inst.ins.sync_info.on_wait.clear()
            for w in waits[:MAX_WAITS]:
                drain_inst.ins.sync_info.on_wait.append(w)
            for i in range(MAX_WAITS, len(waits), MAX_WAITS):
                n = nc.sync.nop(nofuse=True)
                n.ins.sync_info = mybir.SyncInfo(
                    on_wait=list(waits[i : i + MAX_WAITS]), on_update=[]
                )
        nc.all_engine_barrier()
        assert self.sems is not None
        popped = nc._tile_sem_poison_stack.pop()
        assert popped is self._sem_poison
        nc.clear_and_free_semaphores(list(self.sems.allocated().values()))
        nc.all_engine_barrier()


def _build_program(a_vals):
    """Build the SPMD Bass program. a_vals[n] = -exp(A_log[0,n])."""
    nc = bass.Bass("TRN2", target_bir_lowering=False, debug=False,
                   num_devices=N_CORES)

    def din(name, shape):
        return nc.dram_tensor(name, shape, f32, kind="ExternalInput").ap()

    def dout(name, shape):
        return nc.dram_tensor(name, shape, f32, kind="ExternalOutput").ap()

    inp = din("inp", [T, TOTAL_IN])
    W_in = din("W_in", [TOTAL_IN, HIDDEN])
    b_in = din("b_in", [HIDDEN])
    in_proj_w = din("in_proj_w", [HIDDEN, 2 * D_INNER])
    conv_w = din("conv_w", [D_INNER, D_CONV])
    conv_b = din("conv_b", [D_INNER])
    x_proj_w = din("x_proj_w", [D_INNER, DT_RANK + 2 * D_STATE])
    dt_proj_w = din("dt_proj_w", [DT_RANK, D_INNER])
    dt_proj_b = din("dt_proj_b", [D_INNER])
    D_param = din("D_param", [D_INNER])
    out_proj_w = din("out_proj_w", [D_INNER, HIDDEN])
    ln_g = din("ln_g", [HIDDEN])
    ln_b = din("ln_b", [HIDDEN])
    mean_w = din("mean_w", [HIDDEN, OUT_DIM])
    mean_b = din("mean_b", [OUT_DIM])
    logvar_w = din("logvar_w", [HIDDEN, OUT_DIM])
    logvar_b = din("logvar_b", [OUT_DIM])
    hawkes_w = din("hawkes_w", [HIDDEN, TREAT_DIM])
    hawkes_bias = din("hawkes_bias", [TREAT_DIM])  # hawkes_base + hawkes_b

    y_pred_o = dout("y_pred", [T, OUT_DIM])
    y_var_o = dout("y_var", [T, OUT_DIM])
    hawkes_o = dout("hawkes", [T, TREAT_DIM])
    h_o = dout("h", [T, HIDDEN])
    S_ha_o = dout("S_ha", [TREAT_DIM, HIDDEN])
    s_h_o = dout("s_h", [HIDDEN])
    q_h_o = dout("q_h", [HIDDEN])

    bc_rows = nc.dram_tensor("bc_rows", [2 * D_STATE, T], bf16).ap()
    ln_rows = nc.dram_tensor("ln_rows", [2, T], f32).ap()

    with SplitWaitTileContext(nc, pool_alloc_mode="queue") as tc, ExitStack() as top:
        const = top.enter_context(tc.tile_pool(name="const", bufs=1))

        ident_b = const.tile([128, 128], bf16)
        make_identity(nc, ident_b)
        ident_f = const.tile([128, 128], f32)
        make_identity(nc, ident_f)
        ident_r = const.tile([128, 128], f32r)
        nc.gpsimd.tensor_copy(ident_r[:], ident_f[:])
        one_b = const.tile([128, 1], f32)
        nc.vector.memset(one_b[:], 1.0)
        ones_r = const.tile([128, 1], f32r)
        nc.gpsimd.tensor_copy(ones_r[:], one_b[:])
        eps_c = const.tile([1, 1], f32)
        nc.vector.memset(eps_c[:], LN_EPS)

        b_in_t = const.tile([128, NH], f32)
        lng_t = const.tile([128, NH], f32)
        lnb_t = const.tile([128, NH], f32)
        for m in range(NH):
            sl = slice(m * 128, (m + 1) * 128)
            nc.sync.dma_start(b_in_t[:, m : m + 1], b_in[sl, None])
            nc.sync.dma_start(lng_t[:, m : m + 1], ln_g[sl, None])
            nc.sync.dma_start(lnb_t[:, m : m + 1], ln_b[sl, None])
        convb_t = const.tile([128, NJ], f32)
        dtb_t = const.tile([128, NJ], f32)
        Dp_t = const.tile([128, NJ], f32)
        for j in range(NJ):
            sl = slice(j * 128, (j + 1) * 128)
            nc.sync.dma_start(convb_t[:, j : j + 1], conv_b[sl, None])
            nc.sync.dma_start(dtb_t[:, j : j + 1], dt_proj_b[sl, None])
            nc.sync.dma_start(Dp_t[:, j : j + 1], D_param[sl, None])
        meanb_t = const.tile([32, 1], f32)
        nc.sync.dma_start(meanb_t[:], mean_b[:, None])
        logvb_t = const.tile([32, 1], f32)
        nc.sync.dma_start(logvb_t[:], logvar_b[:, None])
        hawkb_t = const.tile([8, 1], f32)
        nc.sync.dma_start(hawkb_t[:], hawkes_bias[:, None])
        convw_t = const.tile([128, NJ * D_CONV], f32)
        for j in range(NJ):
            nc.sync.dma_start(
                convw_t[:, j * D_CONV : (j + 1) * D_CONV],
                conv_w[j * 128 : (j + 1) * 128, :],
            )

        # long-lived activation stores
        live = top.enter_context(tc.tile_pool(name="live", bufs=1))
        xdbl_sb = live.tile([64, T], f32r)
        dtw_r = live.tile([32, D_INNER], f32r)
        uz_cm = tc.tile_pool(name="uz", bufs=1, side="right")
        uz_p = uz_cm.__enter__()
        uTb = uz_p.tile([128, NJ * T], bf16)    # silu(conv(xi)), bf16
        zsT = uz_p.tile([128, NJ * T], bf16)    # silu(z)

        # ================= phases 0-2: input proj, xz, conv, x_dbl ========
        with tc.tile_pool(name="ph01", bufs=1) as ph01, \
             tc.tile_pool(name="tp_sb", bufs=3) as tp_sb, \
             tc.tile_pool(name="tp_ps", bufs=2, space="PSUM") as tp_ps, \
             tc.tile_pool(name="gem_ps", bufs=2, space="PSUM") as gem_ps, \
             tc.tile_pool(name="xdbl_ps", bufs=1, space="PSUM") as xdbl_ps, \
             tc.tile_pool(name="wtmp", bufs=1) as wtmp, \
             tc.tile_pool(name="conv_p", bufs=2) as conv_p:

            # inp -> inpT [104, T] f32r
            inpT = ph01.tile([TOTAL_IN, T], f32r)
            for i in range(T // 128):
                it = tp_sb.tile([128, TOTAL_IN], f32, tag="inp_in")
                nc.sync.dma_start(it[:], inp[i * 128 : (i + 1) * 128, :])
                ps = tp_ps.tile([TOTAL_IN, 128], f32, tag="inp_ps")
                nc.tensor.transpose(ps[:], it[:], ident_f[:])
                nc.scalar.copy(inpT[:, i * 128 : (i + 1) * 128], ps[:])

            # projT = (inp @ W_in + b_in).T stored [128, NH*T] f32r
            W_in_f = wtmp.tile([TOTAL_IN, HIDDEN], f32, tag="wf")
            nc.sync.dma_start(W_in_f[:], W_in[:])
            W_in_r = ph01.tile([TOTAL_IN, HIDDEN], f32r)
            nc.gpsimd.tensor_copy(W_in_r[:], W_in_f[:])
            projT = ph01.tile([128, NH * T], f32r)
            for m in range(NH):
                for hf in range(2):
                    ps = gem_ps.tile([128, 512], f32, tag="g1")
                    nc.tensor.matmul(
                        ps[:],
                        W_in_r[:, m * 128 : (m + 1) * 128],
                        inpT[:, hf * 512 : (hf + 1) * 512],
                        start=True, stop=True,
                    )
                    nc.scalar.activation(
                        projT[:, m * T + hf * 512 : m * T + (hf + 1) * 512],
                        ps[:], AF.Identity, bias=b_in_t[:, m : m + 1],
                    )

            # weights for xz / x_dbl
            ipw_r = []
            for k in range(NH):
                wf = wtmp.tile([128, 2 * D_INNER], f32, tag="ipwf")
                nc.sync.dma_start(wf[:], in_proj_w[k * 128 : (k + 1) * 128, :])
                wr = ph01.tile([128, 2 * D_INNER], f32r, tag=f"ipwr{k}")
                nc.gpsimd.tensor_copy(wr[:], wf[:])
                ipw_r.append(wr)
            xpw_f = wtmp.tile([128, NJ * 64], f32, tag="xpwf")
            for k in range(NJ):
                nc.sync.dma_start(
                    xpw_f[:, k * 64 : (k + 1) * 64],
                    x_proj_w[k * 128 : (k + 1) * 128, :],
                )
            xpw_b = ph01.tile([128, NJ * 64], bf16)
            nc.gpsimd.tensor_copy(xpw_b[:], xpw_f[:])

            xdbl_psum = xdbl_ps.tile([64, T], f32)

            for j in range(NJ):
                xi_pad = conv_p.tile([128, T + 3], f32, tag="xi_pad")
                nc.vector.memset(xi_pad[:, 0:3], 0.0)
                for hf in range(2):
                    ps = gem_ps.tile([128, 512], f32, tag="xz")
                    for k in range(NH):
                        nc.tensor.matmul(
                            ps[:],
                            ipw_r[k][:, j * 128 : (j + 1) * 128],
                            projT[:, k * T + hf * 512 : k * T + (hf + 1) * 512],
                            start=(k == 0), stop=(k == NH - 1),
                        )
                    nc.scalar.copy(
                        xi_pad[:, 3 + hf * 512 : 3 + (hf + 1) * 512], ps[:]
                    )
                for hf in range(2):
                    ps = gem_ps.tile([128, 512], f32, tag="xz")
                    for k in range(NH):
                        nc.tensor.matmul(
                            ps[:],
                            ipw_r[k][:, (NJ + j) * 128 : (NJ + j + 1) * 128],
                            projT[:, k * T + hf * 512 : k * T + (hf + 1) * 512],
                            start=(k == 0), stop=(k == NH - 1),
                        )
                    nc.scalar.activation(
                        zsT[:, j * T + hf * 512 : j * T + (hf + 1) * 512],
                        ps[:], AF.Silu,
                    )
                # depthwise causal conv + silu
                xc = conv_p.tile([128, T], f32, tag="xc")
                nc.vector.tensor_scalar(
                    xc[:], xi_pad[:, 0:T],
                    convw_t[:, j * D_CONV : j * D_CONV + 1], None, ALU.mult,
                )
                for k in range(1, D_CONV):
                    xc2 = conv_p.tile([128, T], f32, tag="xc")
                    nc.vector.scalar_tensor_tensor(
                        xc2[:], xi_pad[:, k : k + T],
                        convw_t[:, j * D_CONV + k : j * D_CONV + k + 1],
                        xc[:], ALU.mult, ALU.add,
                    )
                    xc = xc2
                nc.scalar.activation(
                    uTb[:, j * T : (j + 1) * T], xc[:], AF.Silu,
                    bias=convb_t[:, j : j + 1],
                )
                for hf in range(2):
                    nc.tensor.matmul(
                        xdbl_psum[:, hf * 512 : (hf + 1) * 512],
                        xpw_b[:, j * 64 : (j + 1) * 64],
                        uTb[:, j * T + hf * 512 : j * T + (hf + 1) * 512],
                        start=(j == 0), stop=(j == NJ - 1),
                        skip_group_check=True,
                    )

            nc.scalar.copy(xdbl_sb[:], xdbl_psum[:])
            bc_sb = tp_sb.tile([32, T], bf16, tag="bc_sb")
            nc.scalar.copy(bc_sb[:], xdbl_psum[32:64, :])
            nc.sync.dma_start(bc_rows[:], bc_sb[:])

            dtw_f = wtmp.tile([32, D_INNER], f32, tag="dtwf")
            nc.sync.dma_start(dtw_f[:], dt_proj_w[:])
            nc.gpsimd.tensor_copy(dtw_r[:], dtw_f[:])

        # ================= phase 3: the selective scan ====================
        ygp_cm = tc.tile_pool(name="yg", bufs=1)
        yg_pool = ygp_cm.__enter__()
        ygT = yg_pool.tile([128, NJ * T], f32r)  # gated scan output
        with tc.tile_pool(name="scan", bufs=2) as scan_p, \
             tc.tile_pool(name="bcast", bufs=1) as bc_p, \
             tc.tile_pool(name="yacc_ps", bufs=2, space="PSUM") as yacc_ps, \
             tc.tile_pool(name="dt_ps", bufs=2, space="PSUM") as dt_ps:

            Bbc = bc_p.tile([128, D_STATE * T], bf16)
            Cbc = bc_p.tile([128, D_STATE * T], bf16)
            for n in range(D_STATE):
                nc.sync.dma_start(
                    Bbc[:, n * T : (n + 1) * T],
                    bc_rows[n, None, :].broadcast_to((128, T)),
                )
                nc.sync.dma_start(
                    Cbc[:, n * T : (n + 1) * T],
                    bc_rows[D_STATE + n, None, :].broadcast_to((128, T)),
                )

            for j in range(NJ):
                dt_psum = dt_ps.tile([128, T], f32, tag="dtg")
                for hf in range(2):
                    nc.tensor.matmul(
                        dt_psum[:, hf * 512 : (hf + 1) * 512],
                        dtw_r[:, j * 128 : (j + 1) * 128],
                        xdbl_sb[0:DT_RANK, hf * 512 : (hf + 1) * 512],
                        start=True, stop=True, skip_group_check=True,
                    )
                e_t = scan_p.tile([128, T], f32, tag="sp_e")
                nc.scalar.activation(
                    e_t[:], dt_psum[:], AF.Exp, bias=dtb_t[:, j : j + 1]
                )
                dt_t = scan_p.tile([128, T], bf16, tag="dt")
                nc.scalar.activation(dt_t[:], e_t[:], AF.Ln, bias=one_b[:])
                dtu_t = scan_p.tile([128, T], bf16, tag="dtu")
                nc.vector.tensor_tensor(
                    dtu_t[:], dt_t[:], uTb[:, j * T : (j + 1) * T], ALU.mult
                )

                yacc = yacc_ps.tile([128, T], f32, tag="yacc")
                for n in range(D_STATE):
                    dA = scan_p.tile([128, T], bf16, tag="dA")
                    nc.scalar.activation(
                        dA[:], dt_t[:], AF.Exp, scale=float(a_vals[n])
                    )
                    bterm = scan_p.tile([128, T], bf16, tag="bterm")
                    nc.vector.tensor_tensor(
                        bterm[:], dtu_t[:], Bbc[:, n * T : (n + 1) * T], ALU.mult
                    )
                    st = scan_p.tile([128, T], bf16, tag="state")
                    nc.vector.tensor_tensor_scan(
                        st[:], dA[:], bterm[:], 0.0, ALU.mult, ALU.add
                    )
                    ym = scan_p.tile([128, T], bf16, tag="ym")
                    nc.vector.tensor_tensor(
                        ym[:], st[:], Cbc[:, n * T : (n + 1) * T], ALU.mult
                    )
                    for hf in range(2):
                        nc.tensor.matmul(
                            yacc[:, hf * 512 : (hf + 1) * 512],
                            ident_b[:],
                            ym[:, hf * 512 : (hf + 1) * 512],
                            start=(n == 0), stop=(n == D_STATE - 1),
                            skip_group_check=True,
                        )
                y2 = scan_p.tile([128, T], bf16, tag="y2")
                nc.vector.scalar_tensor_tensor(
                    y2[:], uTb[:, j * T : (j + 1) * T], Dp_t[:, j : j + 1],
                    yacc[:], ALU.mult, ALU.add,
                )
                nc.vector.tensor_tensor(
                    ygT[:, j * T : (j + 1) * T], y2[:], zsT[:, j * T : (j + 1) * T],
                    ALU.mult,
                )

        uz_cm.__exit__(None, None, None)

        # ================= phase 4: out_proj =============================
        hpre_cm = tc.tile_pool(name="hpre", bufs=1, side="right")
        hpre_pool = hpre_cm.__enter__()
        hpreT = hpre_pool.tile([128, NH * T], f32r)
        with tc.tile_pool(name="opw", bufs=1) as opw_p, \
             tc.tile_pool(name="opw_tmp", bufs=2) as opw_tmp, \
             tc.tile_pool(name="op_ps", bufs=4, space="PSUM") as op_ps:
            opw_r = []
            for k in range(NJ):
                wf = opw_tmp.tile([128, HIDDEN], f32, tag="opwf")
                nc.sync.dma_start(wf[:], out_proj_w[k * 128 : (k + 1) * 128, :])
                wr = opw_p.tile([128, HIDDEN], f32r, tag=f"opwr{k}")
                nc.gpsimd.tensor_copy(wr[:], wf[:])
                opw_r.append(wr)
            for m in range(NH):
                for hf in range(2):
                    ps = op_ps.tile([128, 512], f32, tag="op")
                    for k in range(NJ):
                        nc.tensor.matmul(
                            ps[:],
                            opw_r[k][:, m * 128 : (m + 1) * 128],
                            ygT[:, k * T + hf * 512 : k * T + (hf + 1) * 512],
                            start=(k == 0), stop=(k == NJ - 1),
                        )
                    nc.scalar.copy(
                        hpreT[:, m * T + hf * 512 : m * T + (hf + 1) * 512], ps[:]
                    )

        ygp_cm.__exit__(None, None, None)

        # ================= phase 5-6: layernorm ==========================
        hln_pool = top.enter_context(tc.tile_pool(name="hln", bufs=1))
        hlnT = hln_pool.tile([128, NH * T], f32r)
        with tc.tile_pool(name="ln_w", bufs=2) as ln_w, \
             tc.tile_pool(name="ln_sb", bufs=1) as ln_sb, \
             tc.tile_pool(name="hsq_p", bufs=1) as hsq_pool, \
             tc.tile_pool(name="ln_ps", bufs=1, space="PSUM") as ln_ps:
            hsq = hsq_pool.tile([128, NH * T], f32r)
            for m in range(NH):
                nc.scalar.activation(
                    hsq[:, m * T : (m + 1) * T], hpreT[:, m * T : (m + 1) * T],
                    AF.Square,
                )
            sum_ps = ln_ps.tile([1, T], f32, tag="s")
            sq_ps = ln_ps.tile([1, T], f32, tag="sq")
            for hf in range(2):
                for m in range(NH):
                    nc.tensor.matmul(
                        sum_ps[:, hf * 512 : (hf + 1) * 512],
                        ones_r[:, 0:1],
                        hpreT[:, m * T + hf * 512 : m * T + (hf + 1) * 512],
                        start=(m == 0), stop=(m == NH - 1),
                        skip_group_check=True,
                    )
                    nc.tensor.matmul(
                        sq_ps[:, hf * 512 : (hf + 1) * 512],
                        ones_r[:, 0:1],
                        hsq[:, m * T + hf * 512 : m * T + (hf + 1) * 512],
                        start=(m == 0), stop=(m == NH - 1),
                        skip_group_check=True,
                    )
            mu = ln_sb.tile([1, T], f32)
            nc.vector.tensor_scalar(mu[:], sum_ps[:], 1.0 / HIDDEN, None, ALU.mult)
            e2 = ln_sb.tile([1, T], f32)
            nc.vector.tensor_scalar(e2[:], sq_ps[:], 1.0 / HIDDEN, None, ALU.mult)
            musq = ln_sb.tile([1, T], f32)
            nc.scalar.activation(musq[:], mu[:], AF.Square)
            var = ln_sb.tile([1, T], f32)
            nc.vector.tensor_tensor(var[:], e2[:], musq[:], ALU.subtract)
            lnv = ln_sb.tile([1, T], f32)
            nc.scalar.activation(lnv[:], var[:], AF.Ln, bias=eps_c[:])
            isd_r = ln_sb.tile([1, T], f32)
            nc.scalar.activation(isd_r[:], lnv[:], AF.Exp, scale=-0.5)
            m2_r = ln_sb.tile([1, T], f32)
            nc.vector.tensor_tensor(m2_r[:], mu[:], isd_r[:], ALU.mult)
            nc.sync.dma_start(ln_rows[0:1, :], isd_r[:])
            nc.sync.dma_start(ln_rows[1:2, :], m2_r[:])
            isd_b = ln_sb.tile([128, T], f32)
            nc.sync.dma_start(isd_b[:], ln_rows[0, None, :].broadcast_to((128, T)))
            m2_b = ln_sb.tile([128, T], f32)
            nc.sync.dma_start(m2_b[:], ln_rows[1, None, :].broadcast_to((128, T)))
            for m in range(NH):
                t1 = ln_w.tile([128, T], f32, tag="t1")
                nc.vector.tensor_tensor(
                    t1[:], hpreT[:, m * T : (m + 1) * T].bitcast(f32), isd_b[:],
                    ALU.mult,
                )
                t2 = ln_w.tile([128, T], f32, tag="t2")
                nc.vector.tensor_tensor(t2[:], t1[:], m2_b[:], ALU.subtract)
                nc.vector.tensor_scalar(
                    hlnT[:, m * T : (m + 1) * T], t2[:],
                    lng_t[:, m : m + 1], lnb_t[:, m : m + 1], ALU.mult, op1=ALU.add,
                )

        hpre_cm.__exit__(None, None, None)

        # ================= phase 7: heads ================================
        ypredT = hln_pool.tile([32, T], f32)
        yvarT = hln_pool.tile([32, T], f32)
        hawkesT = hln_pool.tile([8, T], f32)
        with tc.tile_pool(name="head_sb", bufs=1) as head_sb, \
             tc.tile_pool(name="head_tmp", bufs=2) as head_tmp, \
             tc.tile_pool(name="head_ps", bufs=1, space="PSUM") as head_ps:

            def head_gemm(w_ap, odim, tag):
                wf = head_tmp.tile([128, NH * odim], f32, tag="hwf")
                for k in range(NH):
                    nc.sync.dma_start(
                        wf[:, k * odim : (k + 1) * odim],
                        w_ap[k * 128 : (k + 1) * 128, :],
                    )
                wr = head_tmp.tile([128, NH * odim], f32r, tag="hwr")
                nc.gpsimd.tensor_copy(wr[:], wf[:])
                ps = head_ps.tile([odim, T], f32, tag=tag)
                for hf in range(2):
                    for k in range(NH):
                        nc.tensor.matmul(
                            ps[:, hf * 512 : (hf + 1) * 512],
                            wr[:, k * odim : (k + 1) * odim],
                            hlnT[:, k * T + hf * 512 : k * T + (hf + 1) * 512],
                            start=(k == 0), stop=(k == NH - 1),
                            skip_group_check=True,
                        )
                return ps

            yp_ps = head_gemm(mean_w, OUT_DIM, "yp")
            nc.scalar.activation(ypredT[:], yp_ps[:], AF.Identity, bias=meanb_t[:])

            yv_ps = head_gemm(logvar_w, OUT_DIM, "yv")
            yv_e = head_sb.tile([32, T], f32)
            nc.scalar.activation(yv_e[:], yv_ps[:], AF.Exp, bias=logvb_t[:])
            yv_sp = head_sb.tile([32, T], f32)
            nc.scalar.activation(yv_sp[:], yv_e[:], AF.Ln, bias=one_b[0:32, :])
            nc.vector.tensor_scalar(yvarT[:], yv_sp[:], 1e-4, None, ALU.add)

            hk_ps = head_gemm(hawkes_w, TREAT_DIM, "hk")
            hk_e = head_sb.tile([8, T], f32)
            nc.scalar.activation(hk_e[:], hk_ps[:], AF.Exp, bias=hawkb_t[:])
            nc.scalar.activation(hawkesT[:], hk_e[:], AF.Ln, bias=one_b[0:8, :])

        # ============= phase 8: transposes + outputs =====================
        if True:
            with tc.tile_pool(name="o_ps", bufs=2, space="PSUM") as o_ps, \
                 tc.tile_pool(name="o_sb", bufs=2) as o_sb:
                for i in range(T // 128):
                    hps = o_ps.tile([128, HIDDEN], f32r, tag="h_ps")
                    for m in range(NH):
                        nc.tensor.transpose(
                            hps[:, m * 128 : (m + 1) * 128],
                            hlnT[:, m * T + i * 128 : m * T + (i + 1) * 128],
                            ident_r[:],
                        )
                    hsb = o_sb.tile([128, HIDDEN], f32, tag="h_sb")
                    nc.scalar.copy(hsb[:], hps[:].bitcast(f32))
                    nc.sync.dma_start(h_o[i * 128 : (i + 1) * 128, :], hsb[:])
                for i in range(T // 128):
                    pps = o_ps.tile([128, 72], f32, tag="y_ps")
                    nc.tensor.transpose(
                        pps[:, 0:32], ypredT[:, i * 128 : (i + 1) * 128],
                        ident_f[0:32, 0:32],
                    )
                    nc.tensor.transpose(
                        pps[:, 32:64], yvarT[:, i * 128 : (i + 1) * 128],
                        ident_f[0:32, 0:32],
                    )
                    nc.tensor.transpose(
                        pps[:, 64:72], hawkesT[:, i * 128 : (i + 1) * 128],
                        ident_f[0:8, 0:8],
                    )
                    psb = o_sb.tile([128, 72], f32, tag="y_sb")
                    nc.scalar.copy(psb[:], pps[:])
                    nc.sync.dma_start(
                        y_pred_o[i * 128 : (i + 1) * 128, :], psb[:, 0:32]
                    )
                    nc.sync.dma_start(
                        y_var_o[i * 128 : (i + 1) * 128, :], psb[:, 32:64]
                    )
                    nc.sync.dma_start(
                        hawkes_o[i * 128 : (i + 1) * 128, :], psb[:, 64:72]
                    )

                # ============= phase 9: CDSP partials ====================
                hf_sb = []
                for i in range(2):
                    fps = o_ps.tile([128, HIDDEN], f32r, tag="hf_ps")
                    for m in range(NH):
                        src = hlnT[:, m * T + i * 512 : m * T + (i + 1) * 512 : 4]
                        nc.tensor.transpose(
                            fps[:, m * 128 : (m + 1) * 128], src,
                            ident_r[:],
                        )
                    fsb = o_sb.tile([128, HIDDEN], f32r, tag=f"hf_sb{i}")
                    nc.scalar.copy(fsb[:], fps[:])
                    hf_sb.append(fsb)
                af_f = o_sb.tile([128, 2 * TREAT_DIM], f32, tag="af_f")
                for i in range(2):
                    nc.sync.dma_start(
                        af_f[:, i * TREAT_DIM : (i + 1) * TREAT_DIM],
                        inp[i * 512 : (i + 1) * 512 : 4, INPUT_DIM * 3 :],
                    )
                af_r = o_sb.tile([128, 2 * TREAT_DIM], f32r, tag="af_r")
                nc.gpsimd.tensor_copy(af_r[:], af_f[:])
                sha_ps = o_ps.tile([TREAT_DIM, HIDDEN], f32, tag="sha")
                for i in range(2):
                    nc.tensor.matmul(
                        sha_ps[:], af_r[:, i * TREAT_DIM : (i + 1) * TREAT_DIM],
                        hf_sb[i][:], start=(i == 0), stop=(i == 1),
                        skip_group_check=True,
                    )
                sha_sb = o_sb.tile([TREAT_DIM, HIDDEN], f32, tag="sha_sb")
                nc.scalar.copy(sha_sb[:], sha_ps[:])
                nc.sync.dma_start(S_ha_o[:], sha_sb[:])

                dump = o_sb.tile([128, T // 4], f32, tag="dump")
                sh_acc = o_sb.tile([128, NH], f32, tag="sh")
                qh_acc = o_sb.tile([128, NH], f32, tag="qh")
                for m in range(NH):
                    nc.scalar.activation(
                        dump[:], hlnT[:, m * T : (m + 1) * T : 4].bitcast(f32),
                        AF.Identity, accum_out=sh_acc[:, m : m + 1],
                    )
                    nc.scalar.activation(
                        dump[:], hlnT[:, m * T : (m + 1) * T : 4].bitcast(f32),
                        AF.Square, accum_out=qh_acc[:, m : m + 1],
                    )
                for m in range(NH):
                    nc.sync.dma_start(
                        s_h_o[m * 128 : (m + 1) * 128], sh_acc[:, m : m + 1]
                    )
                    nc.sync.dma_start(
                        q_h_o[m * 128 : (m + 1) * 128], qh_acc[:, m : m + 1]
                    )

    return nc


_PROGRAM_CACHE = {}
LAST_RESULT = None


def kernel(**inputs):
    x_seq = np.asarray(inputs["x_seq"], np.float32)
    mask_seq = np.asarray(inputs["mask_seq"], np.float32)
    dt_seq = np.asarray(inputs["dt_seq"], np.float32)
    a_seq = np.asarray(inputs["a_seq"], np.float32)
    A_log = np.asarray(inputs["A_log"], np.float32)

    inp_all = np.concatenate([x_seq, mask_seq, dt_seq, a_seq], axis=-1)  # [B,T,104]
    a_vals = tuple((-np.exp(A_log[0].astype(np.float64))).tolist())

    if a_vals not in _PROGRAM_CACHE:
        _PROGRAM_CACHE[a_vals] = _build_program(a_vals)
    nc = _PROGRAM_CACHE[a_vals]

    shared = {
        "W_in": inputs["W_in"], "b_in": inputs["b_in"],
        "in_proj_w": inputs["in_proj_w"],
        "conv_w": inputs["conv_w"], "conv_b": inputs["conv_b"],
        "x_proj_w": inputs["x_proj_w"],
        "dt_proj_w": inputs["dt_proj_w"], "dt_proj_b": inputs["dt_proj_b"],
        "D_param": inputs["D_param"], "out_proj_w": inputs["out_proj_w"],
        "ln_g": inputs["ln_g"], "ln_b": inputs["ln_b"],
        "mean_w": inputs["mean_w"], "mean_b": inputs["mean_b"],
        "logvar_w": inputs["logvar_w"], "logvar_b": inputs["logvar_b"],
        "hawkes_w": inputs["hawkes_w"],
        "hawkes_bias": np.asarray(inputs["hawkes_base"], np.float32)
        + np.asarray(inputs["hawkes_b"], np.float32),
    }
    shared = {k: np.ascontiguousarray(v, dtype=np.float32) for k, v in shared.items()}
    in_maps = [dict(shared, inp=np.ascontiguousarray(inp_all[b])) for b in range(B)]

    trace = bool(os.environ.get("BASS_KERNEL_TRACE"))
    res = run_bass_kernel_spmd(nc, in_maps, list(range(N_CORES)), trace=trace)
    global LAST_RESULT
    LAST_RESULT = res
    outs = res.results

    y_pred = np.stack([outs[b]["y_pred"] for b in range(B)])
    y_var = np.stack([outs[b]["y_var"] for b in range(B)])
    hawkes = np.stack([outs[b]["hawkes"] for b in range(B)])
    h = np.stack([outs[b]["h"] for b in range(B)])

    # CDSP from per-core sufficient statistics + host-side a statistics
    S_ha = np.sum([outs[b]["S_ha"] for b in range(B)], axis=0).T.astype(np.float64)
    s_h = np.sum([outs[b]["s_h"] for b in range(B)], axis=0).astype(np.float64)
    q_h = np.sum([outs[b]["q_h"] for b in range(B)], axis=0).astype(np.float64)
    a_f = a_seq[:, ::4, :].reshape(-1, TREAT_DIM).astype(np.float64)
    n_rows = a_f.shape[0]
    mu_h = s_h / n_rows
    mu_a = a_f.mean(0)
    cross = S_ha - n_rows * np.outer(mu_h, mu_a)
    nh2 = q_h - n_rows * mu_h**2
    norm_h = np.sqrt(np.maximum(nh2, 0.0))
    a_c = a_f - mu_a
    norm_a = np.sqrt((a_c**2).sum(0))
    corr = cross / ((norm_h[:, None] + 1e-6) * (norm_a[None, :] + 1e-6))
    cdsp = np.float32((corr**2).sum() / (HIDDEN * TREAT_DIM))

    return (y_pred, y_var, hawkes, cdsp, h)
